# revision 30
# baseline (speedup 1.0000x reference)
"""AdaptiveEdgeSmoothing Trainium2 kernel.

Reference semantics (per sample, 1024x1024 f32 image):
    edges     = |conv3x3(mask, LAPLACIAN)|          (SAME zero pad)
    edge_mask = edges > 0.5*edge_sensitivity
    sm        = mask*(1-bf) + box5(mask)/25*bf,  bf = blur_strength/3
    result    = where(edge_mask, sm, mask)
    out       = (result > final_threshold).astype(f32)

Strategy: B=16 samples sharded 2-per-core across 8 NeuronCores (pure data
parallel).  Per core, each image is processed in 9 row-tiles (rows on
partitions, cols on the free axis).  All convolution arithmetic runs on the
TensorEngine as banded fp32r matmuls over column-shifted rhs views of
zero-margined SBUF blocks:
    PSUM1 = 9x - box3(x)            (3 accumulating passes; the Laplacian)
    PSUM2 = (bf/25)*box5(x)+(1-bf)x (5 passes; the smoothed value)
Vertical band weights (incl. SAME-pad clipping and the per-sample bf
scaling) are precomputed in numpy and DMA'd in.  Halo rows are parked at
spare partitions so output rows start at partition 0 on every operand.
Row-tiles are packed side by side in the free axis of big per-image SBUF
buffers so that loads and stores are a few >1MiB SWDGE (gpsimd) DMAs,
which spread across all 16 SDMA engines (HWDGE transfers chunk
32-partitions-per-engine and cap at ~4 engines).  Elementwise tail: ACT
computes Relu(|lap| - thr) as an edge mask (nonzero = edge), DVE
copy_predicated overwrites a copy of x with sm where masked, then one
is_gt against final_threshold writes the packed output block.
"""

import sys

if '/opt/trn_rl_repo' not in sys.path:
    sys.path.insert(0, '/opt/trn_rl_repo')

import numpy as np

import concourse.bass as bass
import concourse.bacc as bacc
import concourse.bass_utils as bass_utils
import concourse.mybir as mybir
from concourse.tile import TileContext, add_dep_helper
from concourse.bass_utils import run_bass_kernel_spmd

# Enable walrus's LDWEIGHTS optimization for this kernel's compile:
# consecutive matmuls sharing a stationary operand skip redundant weight
# loads.  (The flag is hardcoded off in bir_verify_and_optimise.)
if not getattr(bass_utils, "_ldw_opt_patched", False):
    _orig_run_command = bass_utils.run_command

    def _run_command_ldw(argv, **kwargs):
        if isinstance(argv, list):
            argv = ["--enable-ldw-opt=true" if a == "--enable-ldw-opt=false"
                    else a for a in argv]
        return _orig_run_command(argv, **kwargs)

    bass_utils.run_command = _run_command_ldw
    bass_utils._ldw_opt_patched = True

H = W = 1024
N_CORES = 8
IMGS_PER_CORE = 2
F32 = mybir.dt.float32
F32R = mybir.dt.float32r
XP = 1028  # padded block pitch (2-col zero margins each side)

# tile geometry: (out_row_start, n_out, K_data, halo_partition_base, var)
# partitions [0, K_data) hold rows [s, s+K_data); partitions
# [halo_base, halo_base+2) hold rows [s-2, s).
TILES = [(124 * t, 124, 126, 126, (0 if t == 0 else 1)) for t in range(8)]
TILES.append((992, 32, 32, 32, 2))


def _band_templates():
    """Per variant: (V3, V5, I) as [128,128] f32, plus (K_total, nout)."""
    out = []
    for var in range(3):
        s, nout, kd, hb, _ = TILES[0 if var == 0 else (1 if var == 1 else 8)]
        v3 = np.zeros((128, 128), np.float32)
        v5 = np.zeros((128, 128), np.float32)
        ident = np.zeros((128, 128), np.float32)
        for k in range(kd):
            for p in range(nout):
                d = k - p
                if abs(d) <= 1:
                    v3[k, p] = 1.0
                if abs(d) <= 2:
                    v5[k, p] = 1.0
                if d == 0:
                    ident[k, p] = 1.0
        if var != 0:  # top halo rows: partition hb+j holds row s-2+j
            for j in range(2):
                for p in range(nout):
                    d = (j - 2) - p
                    if abs(d) <= 1:
                        v3[hb + j, p] = 1.0
                    if abs(d) <= 2:
                        v5[hb + j, p] = 1.0
        k_tot = 128 if var != 2 else 34
        out.append((v3, v5, ident, k_tot, nout))
    return out


_TEMPLATES = _band_templates()

_compiled = None
last_results = None


def _margin_memsets(nc, blk, nblocks):
    """Zero the 2-col margins of every 1028-wide block in `blk`."""
    nc.vector.memset(blk[:, 0:2].bitcast(F32), 0)
    if nblocks > 1:
        # right margin of block t + left margin of block t+1 are contiguous
        spans = blk[:, 1026:1026 + (nblocks - 1) * XP].rearrange(
            "p (t c) -> p t c", c=XP)[:, :, 0:4]
        nc.vector.memset(spans.bitcast(F32), 0)
    nc.vector.memset(
        blk[:, nblocks * XP - 2:nblocks * XP].bitcast(F32), 0)


def _build():
    nc = bacc.Bacc("TRN2", target_bir_lowering=False, debug=False,
                   num_devices=N_CORES)
    x = nc.dram_tensor("x", [IMGS_PER_CORE, H, W], F32R,
                       kind="ExternalInput")
    w3p = nc.dram_tensor("w3p", [128, 3 * 2 * 128], F32R,
                         kind="ExternalInput").ap()
    w5p = nc.dram_tensor("w5p", [128, IMGS_PER_CORE * 3 * 2 * 128], F32R,
                         kind="ExternalInput").ap()
    negthr = nc.dram_tensor("negthr", [IMGS_PER_CORE, 128, 1], F32,
                            kind="ExternalInput").ap()
    ft = nc.dram_tensor("ft", [IMGS_PER_CORE, 128, 1], F32,
                        kind="ExternalInput").ap()
    y = nc.dram_tensor("out", [IMGS_PER_CORE, H, W], mybir.dt.uint8,
                       kind="ExternalOutput")

    def xdma(img, out_ap, row0, nrows, ntiles):
        """DRAM read AP: partition p, block t -> image row row0 + 124t + p."""
        return nc.gpsimd.dma_start(
            out=out_ap,
            in_=bass.AP(x, img * H * W + row0 * W,
                        [[W, nrows], [124 * W, ntiles], [1, W]]))

    with TileContext(nc) as tc:
        with (
            tc.tile_pool(name="wpool", bufs=1) as wpool,
            tc.tile_pool(name="spool", bufs=1) as spool,
            tc.tile_pool(name="xpool", bufs=1) as xpool,
            tc.tile_pool(name="p1pool", bufs=2, space="PSUM") as p1pool,
            tc.tile_pool(name="p2pool", bufs=2, space="PSUM") as p2pool,
            tc.tile_pool(name="apool", bufs=3) as apool,
            tc.tile_pool(name="empool", bufs=3) as empool,
            tc.tile_pool(name="vpool", bufs=3) as vpool,
            tc.tile_pool(name="upool", bufs=4) as upool,
            tc.tile_pool(name="opool", bufs=6) as opool,
        ):
            # --- one-time loads: weights + per-image scalars -------------
            # (HWDGE so they don't queue ahead of image data on the SWDGE
            # rings)
            w3all = wpool.tile([128, 3 * 2 * 128], F32R, tag="w3all")
            nc.scalar.dma_start(out=w3all[:], in_=w3p)
            w5all = wpool.tile([128, IMGS_PER_CORE * 3 * 2 * 128], F32R,
                               tag="w5all")
            # per-image halves so image 0's weights land first
            nc.scalar.dma_start(out=w5all[:, 0:768], in_=w5p[:, 0:768])
            nc.scalar.dma_start(out=w5all[:, 768:1536], in_=w5p[:, 768:1536])

            def w3_ap(v, sc):
                return w3all[:, (v * 2 + sc) * 128:(v * 2 + sc) * 128 + 128]

            def w5_ap(img, v, sc):
                base = ((img * 3 + v) * 2 + sc) * 128
                return w5all[:, base:base + 128]

            sc_t = []
            for img in range(IMGS_PER_CORE):
                nt = spool.tile([128, 1], F32, tag=f"nt{img}")
                f = spool.tile([128, 1], F32, tag=f"ft{img}")
                nc.scalar.dma_start(out=nt[:], in_=negthr[img])
                nc.scalar.dma_start(out=f[:], in_=ft[img])
                sc_t.append((nt, f))

            def emit_group(img, k, gate=None):
                """Load 2-block group k (tiles 2k, 2k+1); k=4 is tile 8."""
                if k < 4:
                    g = xpool.tile([128, 2 * XP], F32R, tag=f"x{img}g{k}")
                    _margin_memsets(nc, g, 2)
                    g3 = g[:, :].rearrange("p (t c) -> p t c", c=XP)
                    s0 = 124 * 2 * k
                    # per-block main loads: finer completion granularity so
                    # the first tile's matmuls start sooner
                    if img == 0 and k == 0:
                        # very first block via sync HWDGE: issues earliest
                        nc.sync.dma_start(out=g3[0:126, 0, 2:1026],
                                          in_=x.ap()[0, 0:126, :])
                        nc.sync.dma_start(out=g3[126:128, 0, 2:1026],
                                          in_=x.ap()[0, 0:2, :])
                    else:
                        ld = xdma(img, g3[0:126, 0:1, 2:1026], s0, 126, 1)
                        if gate is not None:
                            add_dep_helper(ld.ins, gate.ins,
                                           reason="stagger")
                        if k == 0:  # t0 halo rows are zero-weight dummies
                            xdma(img, g3[126:128, 0:1, 2:1026], 0, 2, 1)
                        else:
                            xdma(img, g3[126:128, 0:1, 2:1026], s0 - 2, 2, 1)
                    xdma(img, g3[0:126, 1:2, 2:1026], s0 + 124, 126, 1)
                    xdma(img, g3[126:128, 1:2, 2:1026], s0 + 122, 2, 1)
                else:
                    g = xpool.tile([128, XP], F32R, tag=f"x{img}g4")
                    _margin_memsets(nc, g, 1)
                    g3 = g[:, :].rearrange("p (t c) -> p t c", c=XP)
                    ld = nc.gpsimd.dma_start(out=g3[0:32, 0, 2:1026],
                                             in_=x.ap()[img, 992:1024, :])
                    if gate is not None:
                        add_dep_helper(ld.ins, gate.ins, reason="stagger")
                    nc.gpsimd.dma_start(out=g3[32:34, 0, 2:1026],
                                        in_=x.ap()[img, 990:992, :])
                return g3

            # --- main loop ----------------------------------------------
            SEQ = list(range(9))
            xg = {(0, 0): emit_group(0, 0)}
            first_mm = None
            tile_mm = {}
            # emitted after tile (img,t): list of (img, group, gate_tile)
            prefetch = {(0, 0): [(0, 1, (0, 0)), (0, 2, (0, 0))],
                        (0, 1): [(0, 3, (0, 1))],
                        (0, 2): [(0, 4, (0, 2))],
                        (0, 3): [(1, 0, (0, 3))],
                        (0, 5): [(1, 1, (0, 5))],
                        (0, 7): [(1, 2, (0, 7))],
                        (1, 0): [(1, 3, (1, 0))],
                        (1, 2): [(1, 4, (1, 2))]}
            for img in range(IMGS_PER_CORE):
                nt_ap, ft_ap = sc_t[img]
                for t in SEQ:
                    s, nout, kd, hb, var = TILES[t]
                    k_tot = _TEMPLATES[var][3]
                    xt3 = xg[(img, min(t // 2, 4))]
                    blk = t % 2 if t < 8 else 0

                    p1 = p1pool.tile([128, 1024], F32, tag="p1")
                    p2 = p2pool.tile([128, 1024], F32, tag="p2")
                    # On alternating tiles, compute the +-2 horizontal
                    # taps as one DVE add (u = x<<2 + x>>2), replacing two
                    # PE passes with one pass over u (PE/DVE balance).
                    # high_priority orders the add ahead of the psum-gated
                    # DVE tail ops so the PE is not stalled.
                    use_u = first_mm is not None
                    u_t = None
                    if use_u:
                        u_t = upool.tile([128, 1024], F32R, tag="u")
                        with tc.high_priority(offset=60):
                            nc.vector.tensor_tensor(
                                u_t[0:k_tot, :],
                                xt3[0:k_tot, blk, 0:1024].bitcast(F32),
                                xt3[0:k_tot, blk, 4:1028].bitcast(F32),
                                mybir.AluOpType.add)
                        w5shifts = (-1, 1, None)
                    else:
                        w5shifts = (-2, -1, 1, 2)
                    groups = [
                        (p1, w3_ap(var, 0), (-1, 1), False),
                        (p1, w3_ap(var, 1), (0,), True),
                        (p2, w5_ap(img, var, 0), w5shifts, False),
                        (p2, w5_ap(img, var, 1), (0,), True),
                    ]
                    for ps, wt, shifts, is_last in groups:
                        first = shifts[0] in (-1, -2)
                        for si, sh in enumerate(shifts):
                            for c in (0, 512):
                                if sh is None:
                                    rhs = u_t[0:k_tot, c:c + 512]
                                else:
                                    rhs = xt3[0:k_tot, blk,
                                              2 + sh + c:2 + sh + c + 512]
                                mm = nc.tensor.matmul(
                                    ps[0:nout, c:c + 512],
                                    wt[0:k_tot, 0:nout],
                                    rhs,
                                    start=(first and si == 0),
                                    stop=is_last)
                                if first_mm is None:
                                    first_mm = mm
                                tile_mm.setdefault((img, t), mm)

                    # edge mask: nonzero where |lap| > thr
                    a_t = apool.tile([128, 1024], F32, tag="a")
                    em_t = empool.tile([128, 1024], F32, tag="em")
                    nc.scalar.activation(a_t[0:nout, :], p1[0:nout, :],
                                         mybir.ActivationFunctionType.Abs)
                    nc.scalar.activation(em_t[0:nout, :], a_t[0:nout, :],
                                         mybir.ActivationFunctionType.Relu,
                                         bias=nt_ap[0:nout, :])
                    # v = x; v <- sm where edge; out-block = (v > ft)
                    v_t = vpool.tile([128, 1024], F32, tag="v")
                    nc.gpsimd.tensor_copy(v_t[0:nout, :],
                                          xt3[0:nout, blk, 2:1026]
                                          .bitcast(F32))
                    nc.vector.copy_predicated(v_t[0:nout, :],
                                              em_t[0:nout, :]
                                              .bitcast(mybir.dt.int32),
                                              p2[0:nout, 0:1024])
                    o_t = opool.tile([128, 1024], mybir.dt.uint8, tag="o")
                    nc.vector.tensor_scalar(o_t[0:nout, :],
                                            v_t[0:nout, :],
                                            ft_ap[0:nout, :], None,
                                            mybir.AluOpType.is_gt)
                    nc.gpsimd.dma_start(out=y.ap()[img, s:s + nout, :],
                                        in_=o_t[0:nout, :])

                    # staggered prefetch: each load group starts only after
                    # an earlier tile's compute has begun, so its packets
                    # don't steal SDMA slots from data needed sooner
                    for job in prefetch.get((img, t), []):
                        jimg, jk, jgate = job
                        xg[(jimg, jk)] = emit_group(jimg, jk,
                                                    tile_mm[jgate])
    nc.compile()
    return nc


def _in_maps(mask, blur_strength, edge_sensitivity, final_threshold):
    mask = np.ascontiguousarray(mask.reshape(16, H, W), np.float32)
    bs = np.asarray(blur_strength, np.float32).reshape(16)
    es = np.asarray(edge_sensitivity, np.float32).reshape(16)
    fts = np.asarray(final_threshold, np.float32).reshape(16)

    w3 = np.zeros((3, 2, 128, 128), np.float32)
    for v, (v3, v5t, ident, k_tot, nout) in enumerate(_TEMPLATES):
        w3[v, 0] = -v3
        w3[v, 1] = 9.0 * ident - v3
    w3p = np.ascontiguousarray(
        w3.transpose(2, 0, 1, 3).reshape(128, 3 * 2 * 128))

    maps = []
    for c in range(N_CORES):
        sel = slice(2 * c, 2 * c + 2)
        w5 = np.zeros((IMGS_PER_CORE, 3, 2, 128, 128), np.float32)
        for i in range(IMGS_PER_CORE):
            bf = bs[2 * c + i] / 3.0
            for v, (v3, v5t, ident, k_tot, nout) in enumerate(_TEMPLATES):
                w5[i, v, 0] = (bf / 25.0) * v5t
                w5[i, v, 1] = (bf / 25.0) * v5t + (1.0 - bf) * ident
        w5p = np.ascontiguousarray(
            w5.transpose(3, 0, 1, 2, 4).reshape(
                128, IMGS_PER_CORE * 3 * 2 * 128))
        negthr = np.zeros((IMGS_PER_CORE, 128, 1), np.float32)
        ftm = np.zeros((IMGS_PER_CORE, 128, 1), np.float32)
        for i in range(IMGS_PER_CORE):
            negthr[i, :, 0] = -(0.5 * es[2 * c + i])
            ftm[i, :, 0] = fts[2 * c + i]
        maps.append({
            "x": np.ascontiguousarray(mask[sel]),
            "w3p": w3p,
            "w5p": w5p,
            "negthr": negthr,
            "ft": ftm,
        })
    return maps


def kernel(mask, blur_strength, edge_sensitivity, final_threshold):
    global _compiled, last_results
    if _compiled is None:
        _compiled = _build()
    maps = _in_maps(mask, blur_strength, edge_sensitivity, final_threshold)
    res = run_bass_kernel_spmd(_compiled, maps, core_ids=list(range(N_CORES)))
    last_results = res
    out = np.empty((16, 1, H, W), np.float32)
    for c in range(N_CORES):
        out[2 * c:2 * c + 2, 0] = res.results[c]["out"]  # u8 {0,1} -> f32
    return out


# revision 31
# speedup vs baseline: 1.8160x; 1.8160x over previous
"""AdaptiveEdgeSmoothing Trainium2 kernel.

Reference semantics (per sample, 1024x1024 f32 image):
    edges     = |conv3x3(mask, LAPLACIAN)|          (SAME zero pad)
    edge_mask = edges > 0.5*edge_sensitivity
    sm        = mask*(1-bf) + box5(mask)/25*bf,  bf = blur_strength/3
    result    = where(edge_mask, sm, mask)
    out       = (result > final_threshold).astype(f32)

Strategy: B=16 samples sharded 2-per-core across 8 NeuronCores (pure data
parallel).  Per core, each image is processed in 9 row-tiles (rows on
partitions, cols on the free axis).  All convolution arithmetic runs on the
TensorEngine as banded fp32r matmuls over column-shifted rhs views of
zero-margined SBUF blocks:
    PSUM1 = 9x - box3(x)            (3 accumulating passes; the Laplacian)
    PSUM2 = (bf/25)*box5(x)+(1-bf)x (5 passes; the smoothed value)
Vertical band weights (incl. SAME-pad clipping and the per-sample bf
scaling) are precomputed in numpy and DMA'd in.  Halo rows are parked at
spare partitions so output rows start at partition 0 on every operand.
Row-tiles are packed side by side in the free axis of big per-image SBUF
buffers so that loads and stores are a few >1MiB SWDGE (gpsimd) DMAs,
which spread across all 16 SDMA engines (HWDGE transfers chunk
32-partitions-per-engine and cap at ~4 engines).  Elementwise tail: ACT
computes Relu(|lap| - thr) as an edge mask (nonzero = edge), DVE
copy_predicated overwrites a copy of x with sm where masked, then one
is_gt against final_threshold writes the packed output block.
"""

import sys

if '/opt/trn_rl_repo' not in sys.path:
    sys.path.insert(0, '/opt/trn_rl_repo')

import numpy as np

import concourse.bass as bass
import concourse.bacc as bacc
import concourse.bass_utils as bass_utils
import concourse.mybir as mybir
from concourse.tile import TileContext, add_dep_helper
from concourse.bass_utils import run_bass_kernel_spmd

# Enable walrus's LDWEIGHTS optimization for this kernel's compile:
# consecutive matmuls sharing a stationary operand skip redundant weight
# loads.  (The flag is hardcoded off in bir_verify_and_optimise.)
if not getattr(bass_utils, "_ldw_opt_patched", False):
    _orig_run_command = bass_utils.run_command

    def _run_command_ldw(argv, **kwargs):
        if isinstance(argv, list):
            argv = ["--enable-ldw-opt=true" if a == "--enable-ldw-opt=false"
                    else a for a in argv]
        return _orig_run_command(argv, **kwargs)

    bass_utils.run_command = _run_command_ldw
    bass_utils._ldw_opt_patched = True

H = W = 1024
N_CORES = 8
IMGS_PER_CORE = 2
F32 = mybir.dt.float32
F32R = mybir.dt.float32r
XP = 1028  # padded block pitch (2-col zero margins each side)

# tile geometry: (out_row_start, n_out, K_data, halo_partition_base, var)
# partitions [0, K_data) hold rows [s, s+K_data); partitions
# [halo_base, halo_base+2) hold rows [s-2, s).
TILES = [(124 * t, 124, 126, 126, (0 if t == 0 else 1)) for t in range(8)]
TILES.append((992, 32, 32, 32, 2))


def _band_templates():
    """Per variant: (V3, V5, I) as [128,128] f32, plus (K_total, nout)."""
    out = []
    for var in range(3):
        s, nout, kd, hb, _ = TILES[0 if var == 0 else (1 if var == 1 else 8)]
        v3 = np.zeros((128, 128), np.float32)
        v5 = np.zeros((128, 128), np.float32)
        ident = np.zeros((128, 128), np.float32)
        for k in range(kd):
            for p in range(nout):
                d = k - p
                if abs(d) <= 1:
                    v3[k, p] = 1.0
                if abs(d) <= 2:
                    v5[k, p] = 1.0
                if d == 0:
                    ident[k, p] = 1.0
        if var != 0:  # top halo rows: partition hb+j holds row s-2+j
            for j in range(2):
                for p in range(nout):
                    d = (j - 2) - p
                    if abs(d) <= 1:
                        v3[hb + j, p] = 1.0
                    if abs(d) <= 2:
                        v5[hb + j, p] = 1.0
        k_tot = 128 if var != 2 else 34
        out.append((v3, v5, ident, k_tot, nout))
    return out


_TEMPLATES = _band_templates()

_compiled = None
last_results = None


def _margin_memsets(nc, blk, nblocks):
    """Zero the 2-col margins of every 1028-wide block in `blk`."""
    nc.vector.memset(blk[:, 0:2].bitcast(F32), 0)
    if nblocks > 1:
        # right margin of block t + left margin of block t+1 are contiguous
        spans = blk[:, 1026:1026 + (nblocks - 1) * XP].rearrange(
            "p (t c) -> p t c", c=XP)[:, :, 0:4]
        nc.vector.memset(spans.bitcast(F32), 0)
    nc.vector.memset(
        blk[:, nblocks * XP - 2:nblocks * XP].bitcast(F32), 0)


def _build():
    nc = bacc.Bacc("TRN2", target_bir_lowering=False, debug=False,
                   num_devices=N_CORES)
    x = nc.dram_tensor("x", [IMGS_PER_CORE, H, W], F32R,
                       kind="ExternalInput")
    w3p = nc.dram_tensor("w3p", [128, 3 * 2 * 128], F32R,
                         kind="ExternalInput").ap()
    w5p = nc.dram_tensor("w5p", [128, IMGS_PER_CORE * 3 * 2 * 128], F32R,
                         kind="ExternalInput").ap()
    negthr = nc.dram_tensor("negthr", [IMGS_PER_CORE, 128, 1], F32,
                            kind="ExternalInput").ap()
    ft = nc.dram_tensor("ft", [IMGS_PER_CORE, 128, 1], F32,
                        kind="ExternalInput").ap()
    y = nc.dram_tensor("out", [IMGS_PER_CORE, H, W], mybir.dt.uint8,
                       kind="ExternalOutput")

    def xdma(img, out_ap, row0, nrows, ntiles):
        """DRAM read AP: partition p, block t -> image row row0 + 124t + p."""
        return nc.gpsimd.dma_start(
            out=out_ap,
            in_=bass.AP(x, img * H * W + row0 * W,
                        [[W, nrows], [124 * W, ntiles], [1, W]]))

    with TileContext(nc) as tc:
        with (
            tc.tile_pool(name="wpool", bufs=1) as wpool,
            tc.tile_pool(name="spool", bufs=1) as spool,
            tc.tile_pool(name="xpool", bufs=1) as xpool,
            tc.tile_pool(name="p1pool", bufs=2, space="PSUM") as p1pool,
            tc.tile_pool(name="p2pool", bufs=2, space="PSUM") as p2pool,
            tc.tile_pool(name="apool", bufs=3) as apool,
            tc.tile_pool(name="empool", bufs=3) as empool,
            tc.tile_pool(name="vpool", bufs=3) as vpool,
            tc.tile_pool(name="upool", bufs=4) as upool,
            tc.tile_pool(name="opool", bufs=6) as opool,
        ):
            # --- one-time loads: weights + per-image scalars -------------
            # (HWDGE so they don't queue ahead of image data on the SWDGE
            # rings)
            w3all = wpool.tile([128, 3 * 2 * 128], F32R, tag="w3all")
            nc.scalar.dma_start(out=w3all[:], in_=w3p)
            w5all = wpool.tile([128, IMGS_PER_CORE * 3 * 2 * 128], F32R,
                               tag="w5all")
            # per-image halves so image 0's weights land first
            nc.scalar.dma_start(out=w5all[:, 0:768], in_=w5p[:, 0:768])
            nc.scalar.dma_start(out=w5all[:, 768:1536], in_=w5p[:, 768:1536])

            def w3_ap(v, sc):
                return w3all[:, (v * 2 + sc) * 128:(v * 2 + sc) * 128 + 128]

            def w5_ap(img, v, sc):
                base = ((img * 3 + v) * 2 + sc) * 128
                return w5all[:, base:base + 128]

            sc_t = []
            for img in range(IMGS_PER_CORE):
                nt = spool.tile([128, 1], F32, tag=f"nt{img}")
                f = spool.tile([128, 1], F32, tag=f"ft{img}")
                nc.scalar.dma_start(out=nt[:], in_=negthr[img])
                nc.scalar.dma_start(out=f[:], in_=ft[img])
                sc_t.append((nt, f))

            def emit_group(img, k, gate=None):
                """Load 2-block group k (tiles 2k, 2k+1); k=4 is tile 8."""
                if k < 4:
                    g = xpool.tile([128, 2 * XP], F32R, tag=f"x{img}g{k}")
                    _margin_memsets(nc, g, 2)
                    g3 = g[:, :].rearrange("p (t c) -> p t c", c=XP)
                    s0 = 124 * 2 * k
                    # per-block main loads: finer completion granularity so
                    # the first tile's matmuls start sooner
                    if img == 0 and k == 0:
                        # very first block via sync HWDGE: issues earliest
                        nc.sync.dma_start(out=g3[0:126, 0, 2:1026],
                                          in_=x.ap()[0, 0:126, :])
                        nc.sync.dma_start(out=g3[126:128, 0, 2:1026],
                                          in_=x.ap()[0, 0:2, :])
                    else:
                        ld = xdma(img, g3[0:126, 0:1, 2:1026], s0, 126, 1)
                        if gate is not None:
                            add_dep_helper(ld.ins, gate.ins,
                                           reason="stagger")
                        if k == 0:  # t0 halo rows are zero-weight dummies
                            xdma(img, g3[126:128, 0:1, 2:1026], 0, 2, 1)
                        else:
                            xdma(img, g3[126:128, 0:1, 2:1026], s0 - 2, 2, 1)
                    xdma(img, g3[0:126, 1:2, 2:1026], s0 + 124, 126, 1)
                    xdma(img, g3[126:128, 1:2, 2:1026], s0 + 122, 2, 1)
                else:
                    g = xpool.tile([128, XP], F32R, tag=f"x{img}g4")
                    _margin_memsets(nc, g, 1)
                    g3 = g[:, :].rearrange("p (t c) -> p t c", c=XP)
                    ld = nc.gpsimd.dma_start(out=g3[0:32, 0, 2:1026],
                                             in_=x.ap()[img, 992:1024, :])
                    if gate is not None:
                        add_dep_helper(ld.ins, gate.ins, reason="stagger")
                    nc.gpsimd.dma_start(out=g3[32:34, 0, 2:1026],
                                        in_=x.ap()[img, 990:992, :])
                return g3

            # --- main loop ----------------------------------------------
            SEQ = list(range(9))
            xg = {(0, 0): emit_group(0, 0)}
            first_mm = None
            tile_mm = {}
            # emitted after tile (img,t): list of (img, group, gate_tile)
            prefetch = {(0, 0): [(0, 1, (0, 0)), (0, 2, (0, 0))],
                        (0, 1): [(0, 3, (0, 1))],
                        (0, 2): [(0, 4, (0, 2))],
                        (0, 3): [(1, 0, (0, 3))],
                        (0, 5): [(1, 1, (0, 5))],
                        (0, 7): [(1, 2, (0, 7))],
                        (1, 0): [(1, 3, (1, 0))],
                        (1, 2): [(1, 4, (1, 2))]}
            for img in range(IMGS_PER_CORE):
                nt_ap, ft_ap = sc_t[img]
                for t in SEQ:
                    s, nout, kd, hb, var = TILES[t]
                    k_tot = _TEMPLATES[var][3]
                    xt3 = xg[(img, min(t // 2, 4))]
                    blk = t % 2 if t < 8 else 0

                    p1 = p1pool.tile([128, 1024], F32, tag="p1")
                    p2 = p2pool.tile([128, 1024], F32, tag="p2")
                    # On alternating tiles, compute the +-2 horizontal
                    # taps as one DVE add (u = x<<2 + x>>2), replacing two
                    # PE passes with one pass over u (PE/DVE balance).
                    # high_priority orders the add ahead of the psum-gated
                    # DVE tail ops so the PE is not stalled.
                    use_u = (img * 9 + t) % 3 != 0 and first_mm is not None
                    u_t = None
                    if use_u:
                        u_t = upool.tile([128, 1024], F32R, tag="u")
                        with tc.high_priority(offset=60):
                            nc.vector.tensor_tensor(
                                u_t[0:k_tot, :],
                                xt3[0:k_tot, blk, 0:1024].bitcast(F32),
                                xt3[0:k_tot, blk, 4:1028].bitcast(F32),
                                mybir.AluOpType.add)
                        w5shifts = (-1, 1, None)
                    else:
                        w5shifts = (-2, -1, 1, 2)
                    groups = [
                        (p1, w3_ap(var, 0), (-1, 1), False),
                        (p1, w3_ap(var, 1), (0,), True),
                        (p2, w5_ap(img, var, 0), w5shifts, False),
                        (p2, w5_ap(img, var, 1), (0,), True),
                    ]
                    for ps, wt, shifts, is_last in groups:
                        first = shifts[0] in (-1, -2)
                        for si, sh in enumerate(shifts):
                            for c in (0, 512):
                                if sh is None:
                                    rhs = u_t[0:k_tot, c:c + 512]
                                else:
                                    rhs = xt3[0:k_tot, blk,
                                              2 + sh + c:2 + sh + c + 512]
                                mm = nc.tensor.matmul(
                                    ps[0:nout, c:c + 512],
                                    wt[0:k_tot, 0:nout],
                                    rhs,
                                    start=(first and si == 0),
                                    stop=is_last)
                                if first_mm is None:
                                    first_mm = mm
                                tile_mm.setdefault((img, t), mm)

                    # edge mask: nonzero where |lap| > thr
                    a_t = apool.tile([128, 1024], F32, tag="a")
                    em_t = empool.tile([128, 1024], F32, tag="em")
                    nc.scalar.activation(a_t[0:nout, :], p1[0:nout, :],
                                         mybir.ActivationFunctionType.Abs)
                    nc.scalar.activation(em_t[0:nout, :], a_t[0:nout, :],
                                         mybir.ActivationFunctionType.Relu,
                                         bias=nt_ap[0:nout, :])
                    # v = x; v <- sm where edge; out-block = (v > ft)
                    v_t = vpool.tile([128, 1024], F32, tag="v")
                    nc.vector.tensor_copy(v_t[0:nout, :],
                                          xt3[0:nout, blk, 2:1026]
                                          .bitcast(F32))
                    nc.vector.copy_predicated(v_t[0:nout, :],
                                              em_t[0:nout, :]
                                              .bitcast(mybir.dt.int32),
                                              p2[0:nout, 0:1024])
                    o_t = opool.tile([128, 1024], mybir.dt.uint8, tag="o")
                    nc.vector.tensor_scalar(o_t[0:nout, :],
                                            v_t[0:nout, :],
                                            ft_ap[0:nout, :], None,
                                            mybir.AluOpType.is_gt)
                    nc.gpsimd.dma_start(out=y.ap()[img, s:s + nout, :],
                                        in_=o_t[0:nout, :])

                    # staggered prefetch: each load group starts only after
                    # an earlier tile's compute has begun, so its packets
                    # don't steal SDMA slots from data needed sooner
                    for job in prefetch.get((img, t), []):
                        jimg, jk, jgate = job
                        xg[(jimg, jk)] = emit_group(jimg, jk,
                                                    tile_mm[jgate])
    nc.compile()
    return nc


def _in_maps(mask, blur_strength, edge_sensitivity, final_threshold):
    mask = np.ascontiguousarray(mask.reshape(16, H, W), np.float32)
    bs = np.asarray(blur_strength, np.float32).reshape(16)
    es = np.asarray(edge_sensitivity, np.float32).reshape(16)
    fts = np.asarray(final_threshold, np.float32).reshape(16)

    w3 = np.zeros((3, 2, 128, 128), np.float32)
    for v, (v3, v5t, ident, k_tot, nout) in enumerate(_TEMPLATES):
        w3[v, 0] = -v3
        w3[v, 1] = 9.0 * ident - v3
    w3p = np.ascontiguousarray(
        w3.transpose(2, 0, 1, 3).reshape(128, 3 * 2 * 128))

    maps = []
    for c in range(N_CORES):
        sel = slice(2 * c, 2 * c + 2)
        w5 = np.zeros((IMGS_PER_CORE, 3, 2, 128, 128), np.float32)
        for i in range(IMGS_PER_CORE):
            bf = bs[2 * c + i] / 3.0
            for v, (v3, v5t, ident, k_tot, nout) in enumerate(_TEMPLATES):
                w5[i, v, 0] = (bf / 25.0) * v5t
                w5[i, v, 1] = (bf / 25.0) * v5t + (1.0 - bf) * ident
        w5p = np.ascontiguousarray(
            w5.transpose(3, 0, 1, 2, 4).reshape(
                128, IMGS_PER_CORE * 3 * 2 * 128))
        negthr = np.zeros((IMGS_PER_CORE, 128, 1), np.float32)
        ftm = np.zeros((IMGS_PER_CORE, 128, 1), np.float32)
        for i in range(IMGS_PER_CORE):
            negthr[i, :, 0] = -(0.5 * es[2 * c + i])
            ftm[i, :, 0] = fts[2 * c + i]
        maps.append({
            "x": np.ascontiguousarray(mask[sel]),
            "w3p": w3p,
            "w5p": w5p,
            "negthr": negthr,
            "ft": ftm,
        })
    return maps


def kernel(mask, blur_strength, edge_sensitivity, final_threshold):
    global _compiled, last_results
    if _compiled is None:
        _compiled = _build()
    maps = _in_maps(mask, blur_strength, edge_sensitivity, final_threshold)
    res = run_bass_kernel_spmd(_compiled, maps, core_ids=list(range(N_CORES)))
    last_results = res
    out = np.empty((16, 1, H, W), np.float32)
    for c in range(N_CORES):
        out[2 * c:2 * c + 2, 0] = res.results[c]["out"]  # u8 {0,1} -> f32
    return out


# revision 33
# speedup vs baseline: 1.8752x; 1.0326x over previous
"""AdaptiveEdgeSmoothing Trainium2 kernel.

Reference semantics (per sample, 1024x1024 f32 image):
    edges     = |conv3x3(mask, LAPLACIAN)|          (SAME zero pad)
    edge_mask = edges > 0.5*edge_sensitivity
    sm        = mask*(1-bf) + box5(mask)/25*bf,  bf = blur_strength/3
    result    = where(edge_mask, sm, mask)
    out       = (result > final_threshold).astype(f32)

Strategy: B=16 samples sharded 2-per-core across 8 NeuronCores (pure data
parallel).  Per core, each image is processed in 9 row-tiles (rows on
partitions, cols on the free axis).  All convolution arithmetic runs on the
TensorEngine as banded fp32r matmuls over column-shifted rhs views of
zero-margined SBUF blocks:
    PSUM1 = 9x - box3(x)            (3 accumulating passes; the Laplacian)
    PSUM2 = (bf/25)*box5(x)+(1-bf)x (5 passes; the smoothed value)
Vertical band weights (incl. SAME-pad clipping and the per-sample bf
scaling) are precomputed in numpy and DMA'd in.  Halo rows are parked at
spare partitions so output rows start at partition 0 on every operand.
Row-tiles are packed side by side in the free axis of big per-image SBUF
buffers so that loads and stores are a few >1MiB SWDGE (gpsimd) DMAs,
which spread across all 16 SDMA engines (HWDGE transfers chunk
32-partitions-per-engine and cap at ~4 engines).  Elementwise tail: ACT
computes Relu(|lap| - thr) as an edge mask (nonzero = edge), DVE
copy_predicated overwrites a copy of x with sm where masked, then one
is_gt against final_threshold writes the packed output block.
"""

import sys

if '/opt/trn_rl_repo' not in sys.path:
    sys.path.insert(0, '/opt/trn_rl_repo')

import numpy as np

import concourse.bass as bass
import concourse.bacc as bacc
import concourse.bass_utils as bass_utils
import concourse.mybir as mybir
from concourse.tile import TileContext, add_dep_helper
from concourse.bass_utils import run_bass_kernel_spmd

# Enable walrus's LDWEIGHTS optimization for this kernel's compile:
# consecutive matmuls sharing a stationary operand skip redundant weight
# loads.  (The flag is hardcoded off in bir_verify_and_optimise.)
if not getattr(bass_utils, "_ldw_opt_patched", False):
    _orig_run_command = bass_utils.run_command

    def _run_command_ldw(argv, **kwargs):
        if isinstance(argv, list):
            argv = ["--enable-ldw-opt=true" if a == "--enable-ldw-opt=false"
                    else a for a in argv]
        return _orig_run_command(argv, **kwargs)

    bass_utils.run_command = _run_command_ldw
    bass_utils._ldw_opt_patched = True

H = W = 1024
N_CORES = 8
IMGS_PER_CORE = 2
F32 = mybir.dt.float32
F32R = mybir.dt.float32r
XP = 1028  # padded block pitch (2-col zero margins each side)

# tile geometry: (out_row_start, n_out, K_data, halo_partition_base, var)
# partitions [0, K_data) hold rows [s, s+K_data); partitions
# [halo_base, halo_base+2) hold rows [s-2, s).
TILES = [(124 * t, 124, 126, 126, (0 if t == 0 else 1)) for t in range(8)]
TILES.append((992, 32, 32, 32, 2))


def _band_templates():
    """Per variant: (V3, V5, I) as [128,128] f32, plus (K_total, nout)."""
    out = []
    for var in range(3):
        s, nout, kd, hb, _ = TILES[0 if var == 0 else (1 if var == 1 else 8)]
        v3 = np.zeros((128, 128), np.float32)
        v5 = np.zeros((128, 128), np.float32)
        ident = np.zeros((128, 128), np.float32)
        for k in range(kd):
            for p in range(nout):
                d = k - p
                if abs(d) <= 1:
                    v3[k, p] = 1.0
                if abs(d) <= 2:
                    v5[k, p] = 1.0
                if d == 0:
                    ident[k, p] = 1.0
        if var != 0:  # top halo rows: partition hb+j holds row s-2+j
            for j in range(2):
                for p in range(nout):
                    d = (j - 2) - p
                    if abs(d) <= 1:
                        v3[hb + j, p] = 1.0
                    if abs(d) <= 2:
                        v5[hb + j, p] = 1.0
        k_tot = 128 if var != 2 else 34
        out.append((v3, v5, ident, k_tot, nout))
    return out


_TEMPLATES = _band_templates()

_compiled = None
last_results = None


def _margin_memsets(nc, blk, nblocks):
    """Zero the 2-col margins of every 1028-wide block in `blk`."""
    nc.vector.memset(blk[:, 0:2].bitcast(F32), 0)
    if nblocks > 1:
        # right margin of block t + left margin of block t+1 are contiguous
        spans = blk[:, 1026:1026 + (nblocks - 1) * XP].rearrange(
            "p (t c) -> p t c", c=XP)[:, :, 0:4]
        nc.vector.memset(spans.bitcast(F32), 0)
    nc.vector.memset(
        blk[:, nblocks * XP - 2:nblocks * XP].bitcast(F32), 0)


def _build():
    nc = bacc.Bacc("TRN2", target_bir_lowering=False, debug=False,
                   num_devices=N_CORES)
    x = nc.dram_tensor("x", [IMGS_PER_CORE, H, W], F32R,
                       kind="ExternalInput")
    w3p = nc.dram_tensor("w3p", [128, 3 * 2 * 128], F32R,
                         kind="ExternalInput").ap()
    w5p = nc.dram_tensor("w5p", [128, IMGS_PER_CORE * 3 * 2 * 128], F32R,
                         kind="ExternalInput").ap()
    negthr = nc.dram_tensor("negthr", [IMGS_PER_CORE, 128, 1], F32,
                            kind="ExternalInput").ap()
    ft = nc.dram_tensor("ft", [IMGS_PER_CORE, 128, 1], F32,
                        kind="ExternalInput").ap()
    y = nc.dram_tensor("out", [IMGS_PER_CORE, H, W], mybir.dt.uint8,
                       kind="ExternalOutput")

    def xdma(img, out_ap, row0, nrows, ntiles):
        """DRAM read AP: partition p, block t -> image row row0 + 124t + p."""
        return nc.gpsimd.dma_start(
            out=out_ap,
            in_=bass.AP(x, img * H * W + row0 * W,
                        [[W, nrows], [124 * W, ntiles], [1, W]]))

    with TileContext(nc) as tc:
        with (
            tc.tile_pool(name="wpool", bufs=1) as wpool,
            tc.tile_pool(name="spool", bufs=1) as spool,
            tc.tile_pool(name="xpool", bufs=1) as xpool,
            tc.tile_pool(name="p1pool", bufs=2, space="PSUM") as p1pool,
            tc.tile_pool(name="p2pool", bufs=2, space="PSUM") as p2pool,
            tc.tile_pool(name="apool", bufs=3) as apool,
            tc.tile_pool(name="empool", bufs=3) as empool,
            tc.tile_pool(name="vpool", bufs=3) as vpool,
            tc.tile_pool(name="upool", bufs=4) as upool,
            tc.tile_pool(name="opool", bufs=6) as opool,
        ):
            # --- one-time loads: weights + per-image scalars -------------
            # (HWDGE so they don't queue ahead of image data on the SWDGE
            # rings)
            w3all = wpool.tile([128, 3 * 2 * 128], F32R, tag="w3all")
            nc.scalar.dma_start(out=w3all[:], in_=w3p)
            w5all = wpool.tile([128, IMGS_PER_CORE * 3 * 2 * 128], F32R,
                               tag="w5all")
            # per-image halves so image 0's weights land first
            nc.scalar.dma_start(out=w5all[:, 0:768], in_=w5p[:, 0:768])
            nc.scalar.dma_start(out=w5all[:, 768:1536], in_=w5p[:, 768:1536])

            def w3_ap(v, sc):
                return w3all[:, (v * 2 + sc) * 128:(v * 2 + sc) * 128 + 128]

            def w5_ap(img, v, sc):
                base = ((img * 3 + v) * 2 + sc) * 128
                return w5all[:, base:base + 128]

            sc_t = []
            for img in range(IMGS_PER_CORE):
                nt = spool.tile([128, 1], F32, tag=f"nt{img}")
                f = spool.tile([128, 1], F32, tag=f"ft{img}")
                nc.scalar.dma_start(out=nt[:], in_=negthr[img])
                nc.scalar.dma_start(out=f[:], in_=ft[img])
                sc_t.append((nt, f))

            def emit_group(img, k, gate=None):
                """Load 2-block group k (tiles 2k, 2k+1); k=4 is tile 8."""
                if k < 4:
                    g = xpool.tile([128, 2 * XP], F32R, tag=f"x{img}g{k}")
                    _margin_memsets(nc, g, 2)
                    g3 = g[:, :].rearrange("p (t c) -> p t c", c=XP)
                    s0 = 124 * 2 * k
                    # per-block main loads: finer completion granularity so
                    # the first tile's matmuls start sooner
                    if img == 0 and k == 0:
                        # very first block via sync HWDGE: issues earliest
                        nc.sync.dma_start(out=g3[0:126, 0, 2:1026],
                                          in_=x.ap()[0, 0:126, :])
                        nc.sync.dma_start(out=g3[126:128, 0, 2:1026],
                                          in_=x.ap()[0, 0:2, :])
                    else:
                        ld = xdma(img, g3[0:126, 0:1, 2:1026], s0, 126, 1)
                        if gate is not None:
                            add_dep_helper(ld.ins, gate.ins,
                                           reason="stagger")
                        if k == 0:  # t0 halo rows are zero-weight dummies
                            xdma(img, g3[126:128, 0:1, 2:1026], 0, 2, 1)
                        else:
                            xdma(img, g3[126:128, 0:1, 2:1026], s0 - 2, 2, 1)
                    xdma(img, g3[0:126, 1:2, 2:1026], s0 + 124, 126, 1)
                    xdma(img, g3[126:128, 1:2, 2:1026], s0 + 122, 2, 1)
                else:
                    g = xpool.tile([128, XP], F32R, tag=f"x{img}g4")
                    _margin_memsets(nc, g, 1)
                    g3 = g[:, :].rearrange("p (t c) -> p t c", c=XP)
                    ld = nc.gpsimd.dma_start(out=g3[0:32, 0, 2:1026],
                                             in_=x.ap()[img, 992:1024, :])
                    if gate is not None:
                        add_dep_helper(ld.ins, gate.ins, reason="stagger")
                    nc.gpsimd.dma_start(out=g3[32:34, 0, 2:1026],
                                        in_=x.ap()[img, 990:992, :])
                return g3

            # --- main loop ----------------------------------------------
            SEQ = list(range(9))
            xg = {(0, 0): emit_group(0, 0)}
            first_mm = None
            tile_mm = {}
            # emitted after tile (img,t): list of (img, group, gate_tile)
            prefetch = {(0, 0): [(0, 1, (0, 0)), (0, 2, (0, 0))],
                        (0, 1): [(0, 3, (0, 1))],
                        (0, 2): [(0, 4, (0, 2))],
                        (0, 3): [(1, 0, (0, 3))],
                        (0, 5): [(1, 1, (0, 5))],
                        (0, 7): [(1, 2, (0, 7))],
                        (1, 0): [(1, 3, (1, 0))],
                        (1, 2): [(1, 4, (1, 2))]}
            for img in range(IMGS_PER_CORE):
                nt_ap, ft_ap = sc_t[img]
                for t in SEQ:
                    s, nout, kd, hb, var = TILES[t]
                    k_tot = _TEMPLATES[var][3]
                    xt3 = xg[(img, min(t // 2, 4))]
                    blk = t % 2 if t < 8 else 0

                    p1 = p1pool.tile([128, 1024], F32, tag="p1")
                    p2 = p2pool.tile([128, 1024], F32, tag="p2")
                    # On alternating tiles, compute the +-2 horizontal
                    # taps as one DVE add (u = x<<2 + x>>2), replacing two
                    # PE passes with one pass over u (PE/DVE balance).
                    # high_priority orders the add ahead of the psum-gated
                    # DVE tail ops so the PE is not stalled.
                    use_u = (img * 9 + t) % 3 != 0 and first_mm is not None
                    u_t = None
                    if use_u:
                        u_t = upool.tile([128, 1024], F32R, tag="u")
                        with tc.high_priority(offset=60):
                            nc.vector.tensor_tensor(
                                u_t[0:k_tot, :],
                                xt3[0:k_tot, blk, 0:1024].bitcast(F32),
                                xt3[0:k_tot, blk, 4:1028].bitcast(F32),
                                mybir.AluOpType.add)
                        w5shifts = (-1, 1, None)
                    else:
                        w5shifts = (-2, -1, 1, 2)
                    groups = [
                        (p1, w3_ap(var, 0), (-1, 1), False),
                        (p1, w3_ap(var, 1), (0,), True),
                        (p2, w5_ap(img, var, 0), w5shifts, False),
                        (p2, w5_ap(img, var, 1), (0,), True),
                    ]
                    for ps, wt, shifts, is_last in groups:
                        first = shifts[0] in (-1, -2)
                        for si, sh in enumerate(shifts):
                            for c in (0, 512):
                                if sh is None:
                                    rhs = u_t[0:k_tot, c:c + 512]
                                else:
                                    rhs = xt3[0:k_tot, blk,
                                              2 + sh + c:2 + sh + c + 512]
                                mm = nc.tensor.matmul(
                                    ps[0:nout, c:c + 512],
                                    wt[0:k_tot, 0:nout],
                                    rhs,
                                    start=(first and si == 0),
                                    stop=is_last)
                                if first_mm is None:
                                    first_mm = mm
                                tile_mm.setdefault((img, t), mm)

                    # edge mask: nonzero where |lap| > thr
                    a_t = apool.tile([128, 1024], F32, tag="a")
                    em_t = empool.tile([128, 1024], F32, tag="em")
                    nc.scalar.activation(a_t[0:nout, :], p1[0:nout, :],
                                         mybir.ActivationFunctionType.Abs)
                    nc.scalar.activation(em_t[0:nout, :], a_t[0:nout, :],
                                         mybir.ActivationFunctionType.Relu,
                                         bias=nt_ap[0:nout, :])
                    # v = x; v <- sm where edge; out-block = (v > ft)
                    v_t = vpool.tile([128, 1024], F32, tag="v")
                    nc.vector.tensor_copy(v_t[0:nout, :],
                                          xt3[0:nout, blk, 2:1026]
                                          .bitcast(F32))
                    nc.vector.copy_predicated(v_t[0:nout, :],
                                              em_t[0:nout, :]
                                              .bitcast(mybir.dt.int32),
                                              p2[0:nout, 0:1024])
                    o_t = opool.tile([128, 1024], mybir.dt.uint8, tag="o")
                    nc.vector.tensor_scalar(o_t[0:nout, :],
                                            v_t[0:nout, :],
                                            ft_ap[0:nout, :], None,
                                            mybir.AluOpType.is_gt)
                    nc.gpsimd.dma_start(out=y.ap()[img, s:s + nout, :],
                                        in_=o_t[0:nout, :])

                    # staggered prefetch: each load group starts only after
                    # an earlier tile's compute has begun, so its packets
                    # don't steal SDMA slots from data needed sooner
                    for job in prefetch.get((img, t), []):
                        jimg, jk, jgate = job
                        xg[(jimg, jk)] = emit_group(jimg, jk,
                                                    tile_mm[jgate])
    nc.compile()
    return nc


def _in_maps(mask, blur_strength, edge_sensitivity, final_threshold):
    mask = np.ascontiguousarray(mask.reshape(16, H, W), np.float32)
    bs = np.asarray(blur_strength, np.float32).reshape(16)
    es = np.asarray(edge_sensitivity, np.float32).reshape(16)
    fts = np.asarray(final_threshold, np.float32).reshape(16)

    w3 = np.zeros((3, 2, 128, 128), np.float32)
    for v, (v3, v5t, ident, k_tot, nout) in enumerate(_TEMPLATES):
        w3[v, 0] = -v3
        w3[v, 1] = 9.0 * ident - v3
    w3p = np.ascontiguousarray(
        w3.transpose(2, 0, 1, 3).reshape(128, 3 * 2 * 128))

    maps = []
    for c in range(N_CORES):
        sel = slice(2 * c, 2 * c + 2)
        w5 = np.zeros((IMGS_PER_CORE, 3, 2, 128, 128), np.float32)
        for i in range(IMGS_PER_CORE):
            bf = bs[2 * c + i] / 3.0
            for v, (v3, v5t, ident, k_tot, nout) in enumerate(_TEMPLATES):
                w5[i, v, 0] = (bf / 25.0) * v5t
                w5[i, v, 1] = (bf / 25.0) * v5t + (1.0 - bf) * ident
        w5p = np.ascontiguousarray(
            w5.transpose(3, 0, 1, 2, 4).reshape(
                128, IMGS_PER_CORE * 3 * 2 * 128))
        negthr = np.zeros((IMGS_PER_CORE, 128, 1), np.float32)
        ftm = np.zeros((IMGS_PER_CORE, 128, 1), np.float32)
        for i in range(IMGS_PER_CORE):
            negthr[i, :, 0] = -(0.5 * es[2 * c + i])
            ftm[i, :, 0] = fts[2 * c + i]
        maps.append({
            "x": np.ascontiguousarray(mask[sel]),
            "w3p": w3p,
            "w5p": w5p,
            "negthr": negthr,
            "ft": ftm,
        })
    return maps


def kernel(mask, blur_strength, edge_sensitivity, final_threshold):
    global _compiled, last_results
    if _compiled is None:
        _compiled = _build()
    maps = _in_maps(mask, blur_strength, edge_sensitivity, final_threshold)
    res = run_bass_kernel_spmd(_compiled, maps, core_ids=list(range(N_CORES)))
    last_results = res
    out = np.empty((16, 1, H, W), np.float32)
    for c in range(N_CORES):
        out[2 * c:2 * c + 2, 0] = res.results[c]["out"]  # u8 {0,1} -> f32
    return out


# revision 34
# speedup vs baseline: 1.9261x; 1.0272x over previous
"""AdaptiveEdgeSmoothing Trainium2 kernel.

Reference semantics (per sample, 1024x1024 f32 image):
    edges     = |conv3x3(mask, LAPLACIAN)|          (SAME zero pad)
    edge_mask = edges > 0.5*edge_sensitivity
    sm        = mask*(1-bf) + box5(mask)/25*bf,  bf = blur_strength/3
    result    = where(edge_mask, sm, mask)
    out       = (result > final_threshold).astype(f32)

Strategy: B=16 samples sharded 2-per-core across 8 NeuronCores (pure data
parallel).  Per core, each image is processed in 9 row-tiles (rows on
partitions, cols on the free axis).  All convolution arithmetic runs on the
TensorEngine as banded fp32r matmuls over column-shifted rhs views of
zero-margined SBUF blocks:
    PSUM1 = 9x - box3(x)            (3 accumulating passes; the Laplacian)
    PSUM2 = (bf/25)*box5(x)+(1-bf)x (5 passes; the smoothed value)
Vertical band weights (incl. SAME-pad clipping and the per-sample bf
scaling) are precomputed in numpy and DMA'd in.  Halo rows are parked at
spare partitions so output rows start at partition 0 on every operand.
Row-tiles are packed side by side in the free axis of big per-image SBUF
buffers so that loads and stores are a few >1MiB SWDGE (gpsimd) DMAs,
which spread across all 16 SDMA engines (HWDGE transfers chunk
32-partitions-per-engine and cap at ~4 engines).  Elementwise tail: ACT
computes Relu(|lap| - thr) as an edge mask (nonzero = edge), DVE
copy_predicated overwrites a copy of x with sm where masked, then one
is_gt against final_threshold writes the packed output block.
"""

import sys

if '/opt/trn_rl_repo' not in sys.path:
    sys.path.insert(0, '/opt/trn_rl_repo')

import numpy as np

import concourse.bass as bass
import concourse.bacc as bacc
import concourse.bass_utils as bass_utils
import concourse.mybir as mybir
from concourse.tile import TileContext, add_dep_helper
from concourse.bass_utils import run_bass_kernel_spmd

# Enable walrus's LDWEIGHTS optimization for this kernel's compile:
# consecutive matmuls sharing a stationary operand skip redundant weight
# loads.  (The flag is hardcoded off in bir_verify_and_optimise.)
if not getattr(bass_utils, "_ldw_opt_patched", False):
    _orig_run_command = bass_utils.run_command

    def _run_command_ldw(argv, **kwargs):
        if isinstance(argv, list):
            argv = ["--enable-ldw-opt=true" if a == "--enable-ldw-opt=false"
                    else a for a in argv]
        return _orig_run_command(argv, **kwargs)

    bass_utils.run_command = _run_command_ldw
    bass_utils._ldw_opt_patched = True

H = W = 1024
N_CORES = 8
IMGS_PER_CORE = 2
F32 = mybir.dt.float32
F32R = mybir.dt.float32r
XP = 1028  # padded block pitch (2-col zero margins each side)

# tile geometry: (out_row_start, n_out, K_data, halo_partition_base, var)
# partitions [0, K_data) hold rows [s, s+K_data); partitions
# [halo_base, halo_base+2) hold rows [s-2, s).
TILES = [(124 * t, 124, 126, 126, (0 if t == 0 else 1)) for t in range(8)]
TILES.append((992, 32, 32, 32, 2))


def _band_templates():
    """Per variant: (V3, V5, I) as [128,128] f32, plus (K_total, nout)."""
    out = []
    for var in range(3):
        s, nout, kd, hb, _ = TILES[0 if var == 0 else (1 if var == 1 else 8)]
        v3 = np.zeros((128, 128), np.float32)
        v5 = np.zeros((128, 128), np.float32)
        ident = np.zeros((128, 128), np.float32)
        for k in range(kd):
            for p in range(nout):
                d = k - p
                if abs(d) <= 1:
                    v3[k, p] = 1.0
                if abs(d) <= 2:
                    v5[k, p] = 1.0
                if d == 0:
                    ident[k, p] = 1.0
        if var != 0:  # top halo rows: partition hb+j holds row s-2+j
            for j in range(2):
                for p in range(nout):
                    d = (j - 2) - p
                    if abs(d) <= 1:
                        v3[hb + j, p] = 1.0
                    if abs(d) <= 2:
                        v5[hb + j, p] = 1.0
        k_tot = 128 if var != 2 else 34
        out.append((v3, v5, ident, k_tot, nout))
    return out


_TEMPLATES = _band_templates()

_compiled = None
last_results = None


def _margin_memsets(nc, blk, nblocks):
    """Zero the 2-col margins of every 1028-wide block in `blk`."""
    nc.vector.memset(blk[:, 0:2].bitcast(F32), 0)
    if nblocks > 1:
        # right margin of block t + left margin of block t+1 are contiguous
        spans = blk[:, 1026:1026 + (nblocks - 1) * XP].rearrange(
            "p (t c) -> p t c", c=XP)[:, :, 0:4]
        nc.vector.memset(spans.bitcast(F32), 0)
    nc.vector.memset(
        blk[:, nblocks * XP - 2:nblocks * XP].bitcast(F32), 0)


def _build():
    nc = bacc.Bacc("TRN2", target_bir_lowering=False, debug=False,
                   num_devices=N_CORES)
    x = nc.dram_tensor("x", [IMGS_PER_CORE, H, W], F32R,
                       kind="ExternalInput")
    w3p = nc.dram_tensor("w3p", [128, 3 * 2 * 128], F32R,
                         kind="ExternalInput").ap()
    w5p = nc.dram_tensor("w5p", [128, IMGS_PER_CORE * 3 * 2 * 128], F32R,
                         kind="ExternalInput").ap()
    negthr = nc.dram_tensor("negthr", [IMGS_PER_CORE, 128, 1], F32,
                            kind="ExternalInput").ap()
    ft = nc.dram_tensor("ft", [IMGS_PER_CORE, 128, 1], F32,
                        kind="ExternalInput").ap()
    y = nc.dram_tensor("out", [IMGS_PER_CORE, H, W], mybir.dt.uint8,
                       kind="ExternalOutput")

    def xdma(img, out_ap, row0, nrows, ntiles):
        """DRAM read AP: partition p, block t -> image row row0 + 124t + p."""
        return nc.gpsimd.dma_start(
            out=out_ap,
            in_=bass.AP(x, img * H * W + row0 * W,
                        [[W, nrows], [124 * W, ntiles], [1, W]]))

    with TileContext(nc) as tc:
        with (
            tc.tile_pool(name="wpool", bufs=1) as wpool,
            tc.tile_pool(name="spool", bufs=1) as spool,
            tc.tile_pool(name="xpool", bufs=1) as xpool,
            tc.tile_pool(name="p1pool", bufs=2, space="PSUM") as p1pool,
            tc.tile_pool(name="p2pool", bufs=2, space="PSUM") as p2pool,
            tc.tile_pool(name="apool", bufs=3) as apool,
            tc.tile_pool(name="empool", bufs=3) as empool,
            tc.tile_pool(name="vpool", bufs=3) as vpool,
            tc.tile_pool(name="upool", bufs=4) as upool,
            tc.tile_pool(name="opool", bufs=6) as opool,
        ):
            # --- one-time loads: weights + per-image scalars -------------
            # (HWDGE so they don't queue ahead of image data on the SWDGE
            # rings)
            w3all = wpool.tile([128, 3 * 2 * 128], F32R, tag="w3all")
            nc.scalar.dma_start(out=w3all[:], in_=w3p)
            w5all = wpool.tile([128, IMGS_PER_CORE * 3 * 2 * 128], F32R,
                               tag="w5all")
            # per-image halves so image 0's weights land first
            nc.scalar.dma_start(out=w5all[:, 0:768], in_=w5p[:, 0:768])
            nc.scalar.dma_start(out=w5all[:, 768:1536], in_=w5p[:, 768:1536])

            def w3_ap(v, sc):
                return w3all[:, (v * 2 + sc) * 128:(v * 2 + sc) * 128 + 128]

            def w5_ap(img, v, sc):
                base = ((img * 3 + v) * 2 + sc) * 128
                return w5all[:, base:base + 128]

            sc_t = []
            for img in range(IMGS_PER_CORE):
                nt = spool.tile([128, 1], F32, tag=f"nt{img}")
                f = spool.tile([128, 1], F32, tag=f"ft{img}")
                nc.scalar.dma_start(out=nt[:], in_=negthr[img])
                nc.scalar.dma_start(out=f[:], in_=ft[img])
                sc_t.append((nt, f))

            def emit_group(img, k, gate=None):
                """Load 2-block group k (tiles 2k, 2k+1); k=4 is tile 8."""
                if k < 4:
                    g = xpool.tile([128, 2 * XP], F32R, tag=f"x{img}g{k}")
                    _margin_memsets(nc, g, 2)
                    g3 = g[:, :].rearrange("p (t c) -> p t c", c=XP)
                    s0 = 124 * 2 * k
                    # per-block main loads: finer completion granularity so
                    # the first tile's matmuls start sooner
                    if img == 0 and k == 0:
                        # very first block via sync HWDGE: issues earliest
                        nc.sync.dma_start(out=g3[0:126, 0, 2:1026],
                                          in_=x.ap()[0, 0:126, :])
                        nc.sync.dma_start(out=g3[126:128, 0, 2:1026],
                                          in_=x.ap()[0, 0:2, :])
                    else:
                        ld = xdma(img, g3[0:126, 0:1, 2:1026], s0, 126, 1)
                        if gate is not None:
                            add_dep_helper(ld.ins, gate.ins,
                                           reason="stagger")
                        if k == 0:  # t0 halo rows are zero-weight dummies
                            xdma(img, g3[126:128, 0:1, 2:1026], 0, 2, 1)
                        else:
                            xdma(img, g3[126:128, 0:1, 2:1026], s0 - 2, 2, 1)
                    xdma(img, g3[0:126, 1:2, 2:1026], s0 + 124, 126, 1)
                    xdma(img, g3[126:128, 1:2, 2:1026], s0 + 122, 2, 1)
                else:
                    g = xpool.tile([128, XP], F32R, tag=f"x{img}g4")
                    _margin_memsets(nc, g, 1)
                    g3 = g[:, :].rearrange("p (t c) -> p t c", c=XP)
                    ld = nc.gpsimd.dma_start(out=g3[0:32, 0, 2:1026],
                                             in_=x.ap()[img, 992:1024, :])
                    if gate is not None:
                        add_dep_helper(ld.ins, gate.ins, reason="stagger")
                    nc.gpsimd.dma_start(out=g3[32:34, 0, 2:1026],
                                        in_=x.ap()[img, 990:992, :])
                return g3

            # --- main loop ----------------------------------------------
            SEQ = list(range(9))
            xg = {(0, 0): emit_group(0, 0)}
            first_mm = None
            tile_mm = {}
            # emitted after tile (img,t): list of (img, group, gate_tile)
            prefetch = {(0, 0): [(0, 1, (0, 0)), (0, 2, (0, 0))],
                        (0, 1): [(0, 3, (0, 1))],
                        (0, 2): [(0, 4, (0, 2))],
                        (0, 3): [(1, 0, (0, 3))],
                        (0, 5): [(1, 1, (0, 5))],
                        (0, 7): [(1, 2, (0, 7))],
                        (1, 0): [(1, 3, (1, 0))],
                        (1, 2): [(1, 4, (1, 2))]}
            for img in range(IMGS_PER_CORE):
                nt_ap, ft_ap = sc_t[img]
                for t in SEQ:
                    s, nout, kd, hb, var = TILES[t]
                    k_tot = _TEMPLATES[var][3]
                    xt3 = xg[(img, min(t // 2, 4))]
                    blk = t % 2 if t < 8 else 0

                    p1 = p1pool.tile([128, 1024], F32, tag="p1")
                    p2 = p2pool.tile([128, 1024], F32, tag="p2")
                    # On alternating tiles, compute the +-2 horizontal
                    # taps as one DVE add (u = x<<2 + x>>2), replacing two
                    # PE passes with one pass over u (PE/DVE balance).
                    # high_priority orders the add ahead of the psum-gated
                    # DVE tail ops so the PE is not stalled.
                    use_u = first_mm is not None
                    u_t = None
                    if use_u:
                        u_t = upool.tile([128, 1024], F32R, tag="u")
                        with tc.high_priority(offset=60):
                            nc.vector.tensor_tensor(
                                u_t[0:k_tot, :],
                                xt3[0:k_tot, blk, 0:1024].bitcast(F32),
                                xt3[0:k_tot, blk, 4:1028].bitcast(F32),
                                mybir.AluOpType.add)
                        w5shifts = (-1, 1, None)
                    else:
                        w5shifts = (-2, -1, 1, 2)
                    groups = [
                        (p1, w3_ap(var, 0), (-1, 1), False),
                        (p1, w3_ap(var, 1), (0,), True),
                        (p2, w5_ap(img, var, 0), w5shifts, False),
                        (p2, w5_ap(img, var, 1), (0,), True),
                    ]
                    for ps, wt, shifts, is_last in groups:
                        first = shifts[0] in (-1, -2)
                        for si, sh in enumerate(shifts):
                            for c in (0, 512):
                                if sh is None:
                                    rhs = u_t[0:k_tot, c:c + 512]
                                else:
                                    rhs = xt3[0:k_tot, blk,
                                              2 + sh + c:2 + sh + c + 512]
                                mm = nc.tensor.matmul(
                                    ps[0:nout, c:c + 512],
                                    wt[0:k_tot, 0:nout],
                                    rhs,
                                    start=(first and si == 0),
                                    stop=is_last)
                                if first_mm is None:
                                    first_mm = mm
                                tile_mm.setdefault((img, t), mm)

                    # edge mask: nonzero where |lap| > thr
                    a_t = apool.tile([128, 1024], F32, tag="a")
                    em_t = empool.tile([128, 1024], F32, tag="em")
                    nc.scalar.activation(a_t[0:nout, :], p1[0:nout, :],
                                         mybir.ActivationFunctionType.Abs)
                    nc.scalar.activation(em_t[0:nout, :], a_t[0:nout, :],
                                         mybir.ActivationFunctionType.Relu,
                                         bias=nt_ap[0:nout, :])
                    # v = x; v <- sm where edge; out-block = (v > ft)
                    v_t = vpool.tile([128, 1024], F32, tag="v")
                    nc.scalar.copy(v_t[0:nout, :],
                                   xt3[0:nout, blk, 2:1026].bitcast(F32))
                    nc.vector.copy_predicated(v_t[0:nout, :],
                                              em_t[0:nout, :]
                                              .bitcast(mybir.dt.int32),
                                              p2[0:nout, 0:1024])
                    o_t = opool.tile([128, 1024], mybir.dt.uint8, tag="o")
                    nc.vector.tensor_scalar(o_t[0:nout, :],
                                            v_t[0:nout, :],
                                            ft_ap[0:nout, :], None,
                                            mybir.AluOpType.is_gt)
                    nc.gpsimd.dma_start(out=y.ap()[img, s:s + nout, :],
                                        in_=o_t[0:nout, :])

                    # staggered prefetch: each load group starts only after
                    # an earlier tile's compute has begun, so its packets
                    # don't steal SDMA slots from data needed sooner
                    for job in prefetch.get((img, t), []):
                        jimg, jk, jgate = job
                        xg[(jimg, jk)] = emit_group(jimg, jk,
                                                    tile_mm[jgate])
    nc.compile()
    return nc


def _in_maps(mask, blur_strength, edge_sensitivity, final_threshold):
    mask = np.ascontiguousarray(mask.reshape(16, H, W), np.float32)
    bs = np.asarray(blur_strength, np.float32).reshape(16)
    es = np.asarray(edge_sensitivity, np.float32).reshape(16)
    fts = np.asarray(final_threshold, np.float32).reshape(16)

    w3 = np.zeros((3, 2, 128, 128), np.float32)
    for v, (v3, v5t, ident, k_tot, nout) in enumerate(_TEMPLATES):
        w3[v, 0] = -v3
        w3[v, 1] = 9.0 * ident - v3
    w3p = np.ascontiguousarray(
        w3.transpose(2, 0, 1, 3).reshape(128, 3 * 2 * 128))

    maps = []
    for c in range(N_CORES):
        sel = slice(2 * c, 2 * c + 2)
        w5 = np.zeros((IMGS_PER_CORE, 3, 2, 128, 128), np.float32)
        for i in range(IMGS_PER_CORE):
            bf = bs[2 * c + i] / 3.0
            for v, (v3, v5t, ident, k_tot, nout) in enumerate(_TEMPLATES):
                w5[i, v, 0] = (bf / 25.0) * v5t
                w5[i, v, 1] = (bf / 25.0) * v5t + (1.0 - bf) * ident
        w5p = np.ascontiguousarray(
            w5.transpose(3, 0, 1, 2, 4).reshape(
                128, IMGS_PER_CORE * 3 * 2 * 128))
        negthr = np.zeros((IMGS_PER_CORE, 128, 1), np.float32)
        ftm = np.zeros((IMGS_PER_CORE, 128, 1), np.float32)
        for i in range(IMGS_PER_CORE):
            negthr[i, :, 0] = -(0.5 * es[2 * c + i])
            ftm[i, :, 0] = fts[2 * c + i]
        maps.append({
            "x": np.ascontiguousarray(mask[sel]),
            "w3p": w3p,
            "w5p": w5p,
            "negthr": negthr,
            "ft": ftm,
        })
    return maps


def kernel(mask, blur_strength, edge_sensitivity, final_threshold):
    global _compiled, last_results
    if _compiled is None:
        _compiled = _build()
    maps = _in_maps(mask, blur_strength, edge_sensitivity, final_threshold)
    res = run_bass_kernel_spmd(_compiled, maps, core_ids=list(range(N_CORES)))
    last_results = res
    out = np.empty((16, 1, H, W), np.float32)
    for c in range(N_CORES):
        out[2 * c:2 * c + 2, 0] = res.results[c]["out"]  # u8 {0,1} -> f32
    return out
